# revision 1
# baseline (speedup 1.0000x reference)
"""CT parallel-beam 2D forward projector on 8 Trainium2 NeuronCores.

v3: low-rank (SVD) factorization of the per-angle cumulative-weight fields.

For each view angle the trapezoid footprint weights at pixel (y,x) depend
only on g = f_xi(x) + f_eta(y) (fractional parts of the separable detector
coordinate).  The cumulative fields Phi_i[y,x] = Phic(i-0.5-g) (i=1,2,3)
are numerically low rank (<= ~36 at 5e-4 abs err, worst angle), so the host
factors the stacked [512, 3*512] field matrix M = [Phi1|Phi2|Phi3] ~= By@Bx
(rank J=64, fp16) per angle, and the device reconstructs the weight fields
with three 512-col PE matmuls per 128-row chunk (~640ns) instead of ~25
vector ops per pixel.

Device pipeline per (angle, 128-row eta-chunk):
  PE  : Phi fields = ByT^T @ Bx -> PSUM fp32 (3x 512-col matmuls); half A
        holds (Phi1|Phi2) column-interleaved (host-permuted Bx), half B Phi3
  ACT : drain PSUM -> SBUF bf16
  DVE : T_i = Phi_i * img via packed bf16 tensor_tensor (2x mode) into pair
        tiles [128,2048] bf16 = lane-interleaved (T1|T2) and (T3|img) (img
        lanes persistent); S-halves (adjacent xi sums for 2-pixel bins) via
        packed tensor_tensor adds
  GPSIMD: 2 indirect_copy gathers move fp32 WORDS (= a bf16 field pair)
        through the host-built monotone xi->bin index stream
  PE  : one-hot eta-binning, 7 signed (field, tap-shift) instances as PSUM
        column offsets, bf16 stride-2 lane views of the gathered pairs
  ACT/GPSIMD: drain R psum -> SBUF, single per-angle DMA out
Host: tiny anti-diagonal collapse R[v',m] -> proj[n], plus direct numpy path
for the two degenerate axis-aligned angles.

SPMD: one program for all 8 cores. Cores 0-3 process "class X" angles
(|cos| >= sin) on img; cores 4-7 process "class Y" angles on img.T. All
per-angle variation (SVD factors, one-hots, gather indices) is input data.
"""

import numpy as np

Nx = Ny = 512
Nu = 768
NTHETA = 180
HALF_U = (Nu - 1) / 2.0
NCORES = 8
import os as _os
APC = int(_os.environ.get("CT_APC", "23"))   # angles per core
NCHUNK = 4        # eta chunks of 128
MPAD = 528        # gather output width in words (W <= 513, padded, mult 16)
RW = 531          # R output width (512 + 19)
VP = 96           # local v' bins per chunk (128*0.7072 < 91)
NV = NCHUNK * VP
ZERO_COL = 1023   # word index of the zero column in each pair buffer
J = int(_os.environ.get("CT_J", "64"))  # SVD rank per angle
B_RECT = 1e-4     # below this min-slope, use the host rect path

_PROGRAM_CACHE = {}


def _np_bf16():
    import ml_dtypes
    return ml_dtypes.bfloat16


# --------------------------------------------------------------------------
# host tables
# --------------------------------------------------------------------------

def _angle_tables(theta_val):
    th = float(theta_val)
    c, s = np.cos(th), np.sin(th)
    ac, asn = abs(c), abs(s)
    A, B = max(ac, asn), min(ac, asn)
    b2 = ac + asn
    cls = 0 if ac >= asn else 1
    a_xi, a_eta = (c, s) if cls == 0 else (s, c)
    z0 = HALF_U - b2 / 2 - 255.5 * (c + s)
    grid = np.arange(512)
    pxi = a_xi * grid + z0
    peta = a_eta * grid
    bxi = np.floor(pxi).astype(np.int64)
    fxi = pxi - bxi
    beta = np.floor(peta).astype(np.int64)
    feta = peta - beta
    q = 1.0 / (2 * A * B) if B > B_RECT else None
    return dict(c=c, s=s, A=A, B=B, b2=b2, q=q, cls=cls,
                bxi=bxi, fxi=fxi, beta=beta, feta=feta)


def _gather_tables(T):
    """xi-binning run-starts and the shared index stream (length MPAD)."""
    bxi = T["bxi"]
    bxi_min = int(bxi.min())
    mloc = bxi - bxi_min
    W = int(mloc.max()) + 1
    xa = np.zeros(W, dtype=np.int64)
    L = np.zeros(W, dtype=np.int64)
    order = np.argsort(mloc, kind="stable")
    sorted_m = mloc[order]
    first = np.searchsorted(sorted_m, np.arange(W), side="left")
    last = np.searchsorted(sorted_m, np.arange(W), side="right")
    for m in range(W):
        idxs = order[first[m]:last[m]]
        n = len(idxs)
        assert 1 <= n <= 2
        xa[m] = idxs.min()
        L[m] = n
        if n == 2:
            assert idxs.max() - idxs.min() == 1

    idx = np.full(MPAD, ZERO_COL, dtype=np.int64)
    msrc = np.arange(0, min(W, MPAD))
    idx[:len(msrc)] = np.where(L[msrc] == 2, 512 + xa[msrc], xa[msrc])
    return dict(bxi_min=bxi_min, W=W, stream=idx)


def _wrap_idx(stream):
    """[MPAD] int -> [128, MPAD//16] uint16 wrapped per 16-partition groups."""
    w = stream.reshape(MPAD // 16, 16).T.astype(np.uint16)
    return np.tile(w, (8, 1))


def _phic(t, A, B):
    q = 1.0 / (2 * A * B)
    r = lambda x: np.square(np.maximum(x, 0.0))
    return q * (r(t) - r(t - B) - r(t - A) + r(t - A - B))


def _angle_factors(T):
    """Rank-J factorization of the stacked cumulative fields.

    Returns By [512, J] fp16 and Bx [J, 1536] fp16 with columns permuted so
    device cols 0:1024 are (Phi1|Phi2) lane-interleaved, 1024:1536 Phi3.
    """
    A, B = T["A"], T["B"]
    g = T["feta"][:, None] + T["fxi"][None, :]          # [512, 512]
    Phi1 = _phic(0.5 - g, A, B)
    Phi2 = _phic(1.5 - g, A, B)
    Phi3 = _phic(2.5 - g, A, B)
    M = np.concatenate([Phi1, Phi2, Phi3], axis=1)      # [512, 1536]
    MMt = M @ M.T
    w, V = np.linalg.eigh(MMt)
    order = np.argsort(w)[::-1][:J]
    w = np.maximum(w[order], 1e-20)
    V = V[:, order]                                     # [512, J]
    s = np.sqrt(np.sqrt(w))                             # sigma^(1/2)
    By = V * s[None, :]
    Bx = (V / s[None, :]).T @ M                         # [J, 1536]
    perm = np.empty(1536, dtype=np.int64)
    cols = np.arange(512)
    perm[0:1024:2] = 0 * 512 + cols
    perm[1:1024:2] = 1 * 512 + cols
    perm[1024:1536] = 2 * 512 + cols
    Bx = Bx[:, perm]
    return By.astype(np.float16), Bx.astype(np.float16)


def _core_inputs(img_layout, angle_list, tables):
    """Build the input map for one core. img_layout: [512,512] f32 [eta,xi]."""
    bf16 = _np_bf16()
    A_ = APC
    img = np.ascontiguousarray(img_layout).astype(np.float32)
    imgc = img.reshape(NCHUNK, 128, 512)
    img2 = np.repeat(imgc, 2, axis=2)                   # [4,128,1024]
    img_b = imgc.astype(bf16).astype(np.float32)
    # pairB initial content: odd F lanes = img, odd S lanes = S(img)
    pairB0 = np.zeros((NCHUNK, 128, 2048), dtype=np.float32)
    pairB0[:, :, 1:1024:2] = img_b
    pairB0[:, :, 1025:2046:2] = img_b[:, :, :-1] + img_b[:, :, 1:]

    ByT_t = np.zeros((A_, J, 512), dtype=np.float16)
    Bx_t = np.zeros((A_, J, 1536), dtype=np.float16)
    oh_t = np.zeros((A_, 128, 2 * NV), dtype=np.float32)
    idx_t = np.zeros((A_, 128, MPAD // 16), dtype=np.uint16)
    meta = []
    for ai, a in enumerate(angle_list):
        T = tables[a]
        G = _gather_tables(T)
        By, Bx = _angle_factors(T)
        ByT_t[ai] = By.T
        Bx_t[ai] = Bx
        beta = T["beta"]
        beta0 = []
        for k in range(NCHUNK):
            sl = slice(k * 128, (k + 1) * 128)
            vloc = beta[sl] - beta[sl].min()
            assert vloc.min() >= 0 and vloc.max() < VP
            oh_t[ai, np.arange(128), k * VP + vloc] = 1.0
            oh_t[ai, np.arange(128), NV + k * VP + vloc] = -1.0
            beta0.append(int(beta[sl].min()))
        idx_t[ai] = _wrap_idx(G["stream"])
        meta.append(dict(angle=a, bxi_min=G["bxi_min"], W=G["W"], beta0=beta0))
    in_map = {
        "img2_t": img2.astype(bf16),
        "imgc_t": imgc.astype(bf16),
        "pairB0_t": pairB0.astype(bf16),
        "ByT_t": ByT_t,
        "Bx_t": Bx_t,
        "oh_t": oh_t.astype(bf16),
        "idx_t": idx_t,
    }
    return in_map, meta


# --------------------------------------------------------------------------
# the bass program (identical for all cores)
# --------------------------------------------------------------------------

def _build_program():
    if "nc" in _PROGRAM_CACHE:
        return _PROGRAM_CACHE["nc"]

    import concourse.bass as bass
    import concourse.tile as tile
    from concourse import bacc, mybir
    from contextlib import ExitStack

    dt = mybir.dt
    ALU = mybir.AluOpType

    nc = bacc.Bacc("TRN2", target_bir_lowering=False, debug=False,
                   num_devices=NCORES)

    img2_t = nc.dram_tensor("img2_t", [NCHUNK, 128, 1024], dt.bfloat16,
                            kind="ExternalInput").ap()
    imgc_t = nc.dram_tensor("imgc_t", [NCHUNK, 128, 512], dt.bfloat16,
                            kind="ExternalInput").ap()
    pairB0_t = nc.dram_tensor("pairB0_t", [NCHUNK, 128, 2048], dt.bfloat16,
                              kind="ExternalInput").ap()
    ByT_t = nc.dram_tensor("ByT_t", [APC, J, 512], dt.float16,
                           kind="ExternalInput").ap()
    Bx_t = nc.dram_tensor("Bx_t", [APC, J, 1536], dt.float16,
                          kind="ExternalInput").ap()
    oh_t = nc.dram_tensor("oh_t", [APC, 128, 2 * NV], dt.bfloat16,
                          kind="ExternalInput").ap()
    idx_t = nc.dram_tensor("idx_t", [APC, 128, MPAD // 16], dt.uint16,
                           kind="ExternalInput").ap()
    r_out = nc.dram_tensor("r_out", [APC, VP, NCHUNK, RW], dt.float32,
                           kind="ExternalOutput").ap()

    # (field, psum column shift, sign); order below ensures the first writer
    # of each PSUM piece covers its full written range
    instances = [(0, 0, +1), (3, 3, +1), (1, 1, +1), (2, 2, +1),
                 (0, 1, -1), (1, 2, -1), (2, 3, -1)]

    with tile.TileContext(nc) as tc, ExitStack() as ctx:
        img_pool = ctx.enter_context(tc.tile_pool(name="img", bufs=1))
        pair_pool = ctx.enter_context(tc.tile_pool(name="pairs", bufs=1))
        tab_pool = ctx.enter_context(tc.tile_pool(name="tabs", bufs=2))
        phi_pool = ctx.enter_context(tc.tile_pool(name="phi", bufs=2))
        g_pool = ctx.enter_context(tc.tile_pool(name="gath", bufs=2))
        o_pool = ctx.enter_context(tc.tile_pool(name="outs", bufs=2))
        psfa_pool = ctx.enter_context(tc.tile_pool(name="psumFa", bufs=2,
                                                   space="PSUM"))
        psfb_pool = ctx.enter_context(tc.tile_pool(name="psumFb", bufs=2,
                                                   space="PSUM"))
        psr_pool = ctx.enter_context(tc.tile_pool(name="psumR", bufs=1,
                                                  space="PSUM"))

        img2_ch, imgc_ch, pairsA, pairsB = [], [], [], []
        for k in range(NCHUNK):
            t = img_pool.tile([128, 1024], dt.bfloat16, tag=f"img2c{k}")
            nc.sync.dma_start(t[:], img2_t[k])
            img2_ch.append(t)
            t = img_pool.tile([128, 512], dt.bfloat16, tag=f"imgcc{k}")
            nc.sync.dma_start(t[:], imgc_t[k])
            imgc_ch.append(t)
            pa = pair_pool.tile([128, 2048], dt.bfloat16, tag=f"pairA{k}")
            nc.vector.memset(pa[:, 2046:2048], 0.0)
            pairsA.append(pa)
            pb = pair_pool.tile([128, 2048], dt.bfloat16, tag=f"pairB{k}")
            nc.sync.dma_start(pb[:], pairB0_t[k])
            pairsB.append(pb)

        for ai in range(APC):
            idxt = tab_pool.tile([128, MPAD // 16], dt.uint16, tag="idx")
            nc.sync.dma_start(idxt[:], idx_t[ai])
            bx = tab_pool.tile([J, 1536], dt.float16, tag="bx")
            nc.sync.dma_start(bx[:], Bx_t[ai])
            byt = tab_pool.tile([J, 512], dt.float16, tag="byt")
            nc.sync.dma_start(byt[:], ByT_t[ai])
            oht = tab_pool.tile([128, 2 * NV], dt.bfloat16, tag="oh")
            nc.sync.dma_start(oht[:], oh_t[ai])

            rout = o_pool.tile([VP, NCHUNK * RW], dt.float32, tag="rout")

            for k in range(NCHUNK):
                byk = byt[:, 128 * k:128 * (k + 1)]
                ohk = oht[:, VP * k:VP * (k + 1)]
                ohnk = oht[:, NV + VP * k:NV + VP * (k + 1)]
                pairA, pairB = pairsA[k], pairsB[k]

                psFa = psfa_pool.tile([128, 1024], dt.float32, tag="psFa")
                nc.tensor.matmul(psFa[:, 0:512], byk, bx[:, 0:512],
                                 start=True, stop=True)
                nc.tensor.matmul(psFa[:, 512:1024], byk, bx[:, 512:1024],
                                 start=True, stop=True)
                psFb = psfb_pool.tile([128, 512], dt.float32, tag="psFb")
                nc.tensor.matmul(psFb[:], byk, bx[:, 1024:1536],
                                 start=True, stop=True)

                phiA = phi_pool.tile([128, 1024], dt.bfloat16, tag="phiA")
                nc.scalar.copy(phiA[:], psFa[:])
                phiB = phi_pool.tile([128, 512], dt.bfloat16, tag="phiB")
                nc.scalar.copy(phiB[:], psFb[:])

                # T fields (packed bf16 TT, 2x mode) and S-halves
                nc.vector.tensor_tensor(pairA[:, 0:1024], phiA[:],
                                        img2_ch[k][:], ALU.mult)
                pairB_evenF = pairB[:].rearrange(
                    "p (w l) -> p w l", l=2)[:, 0:512, 0]
                nc.vector.tensor_tensor(pairB_evenF, phiB[:],
                                        imgc_ch[k][:], ALU.mult)
                nc.vector.tensor_tensor(pairA[:, 1024:2046], pairA[:, 0:1022],
                                        pairA[:, 2:1024], ALU.add)
                nc.vector.tensor_tensor(pairB[:, 1024:2046], pairB[:, 0:1022],
                                        pairB[:, 2:1024], ALU.add)

                # gathers: fp32-word views move bf16 field pairs
                gts = []
                for h, pt in enumerate((pairA, pairB)):
                    gt = g_pool.tile([128, MPAD], dt.float32, tag=f"g{h}")
                    nc.gpsimd.indirect_copy(gt[:], pt[:].bitcast(dt.float32),
                                            idxt[:], True)
                    gts.append(gt)

                lanes = []
                for h in range(2):
                    v = gts[h][:].bitcast(dt.bfloat16).rearrange(
                        "p (w l) -> p w l", l=2)
                    lanes.append(v[:, :, 0])
                    lanes.append(v[:, :, 1])

                # one-hot eta-binning; tap shift s as PSUM column offset
                ps = psr_pool.tile([VP, 544], dt.float32, tag="ps")
                mms = []
                for f, s, sgn in instances:
                    lhs = ohk if sgn > 0 else ohnk
                    mms.append((0, ps[:, s:512], lhs, lanes[f][:, 0:512 - s]))
                    mms.append((1, ps[:, 512:528 + s], lhs,
                                lanes[f][:, 512 - s:528]))
                order = [0, 3, 1, 2] + list(range(4, 14))
                started = {0: False, 1: False}
                last_pos = {t: max(p for p, m in enumerate(order)
                                   if mms[m][0] == t) for t in (0, 1)}
                for pos, mi in enumerate(order):
                    tid, out_ap, lhs, rhs_ap = mms[mi]
                    nc.tensor.matmul(out_ap, lhs, rhs_ap,
                                     start=not started[tid],
                                     stop=(pos == last_pos[tid]))
                    started[tid] = True

                nc.scalar.copy(rout[:, RW * k:RW * k + 512], ps[:, 0:512])
                nc.scalar.copy(rout[:, RW * k + 512:RW * (k + 1)],
                               ps[:, 512:531])

            nc.sync.dma_start(r_out[ai], rout[:])

    nc.compile()
    _PROGRAM_CACHE["nc"] = nc
    return nc


# --------------------------------------------------------------------------
# host-side rect path (degenerate angles) — numpy port of the reference
# --------------------------------------------------------------------------

def _host_project(img, theta_vals):
    y = (np.arange(Ny) - (Ny - 1) / 2.0)
    x = (np.arange(Nx) - (Nx - 1) / 2.0)
    y2d, x2d = np.meshgrid(y, x, indexing="ij")
    img_v = img.reshape(-1).astype(np.float64)
    out = np.zeros((len(theta_vals), Nu), dtype=np.float64)
    K = 4
    for t, th in enumerate(theta_vals):
        th = float(th)
        cos_t, sin_t = np.cos(th), np.sin(th)
        ac, asn = abs(cos_t), abs(sin_t)
        h = min(1.0 / ac if ac > 0 else np.inf, 1.0 / asn if asn > 0 else np.inf)
        b1 = abs(asn - ac)
        b2 = abs(asn + ac)
        u0 = x2d * cos_t + y2d * sin_t
        u1 = u0 - b2 / 2
        u2 = u0 - b1 / 2
        u3 = u0 + b1 / 2
        u4 = u0 + b2 / 2
        base = np.floor(u1 + HALF_U).astype(np.int64)
        den12 = (u2 - u1) + (u1 == u2)
        den34 = (u4 - u3) + (u3 == u4)
        acc = np.zeros(Nu + 8, dtype=np.float64)
        for k in range(K):
            idx = base + k
            u = idx - HALF_U
            lo, hi = u - 0.5, u + 0.5
            uA = np.maximum(u1, lo); uB = np.minimum(u2, hi)
            w = (uB > uA) * (h / (2.0 * den12)) * ((uB - u1) ** 2 - (uA - u1) ** 2)
            uA = np.maximum(u2, lo); uB = np.minimum(u3, hi)
            w = w + (uB > uA) * h * (uB - uA)
            uA = np.maximum(u3, lo); uB = np.minimum(u4, hi)
            w = w + (uB > uA) * (h / (2.0 * den34)) * ((uA - u4) ** 2 - (uB - u4) ** 2)
            np.add.at(acc, np.clip(idx.reshape(-1), 0, Nu - 1),
                      img_v * w.reshape(-1))
        out[t] = acc[:Nu]
    return out.astype(np.float32)


# --------------------------------------------------------------------------
# main entry
# --------------------------------------------------------------------------

def kernel(img, theta):
    img = np.asarray(img, dtype=np.float32)
    theta = np.asarray(theta, dtype=np.float32)
    assert img.shape == (Ny, Nx) and theta.shape == (NTHETA,)

    tables = {a: _angle_tables(theta[a]) for a in range(NTHETA)}
    rect_angles = [a for a in range(NTHETA) if tables[a]["q"] is None]
    dev_angles = [a for a in range(NTHETA) if tables[a]["q"] is not None]
    clsX = [a for a in dev_angles if tables[a]["cls"] == 0]
    clsY = [a for a in dev_angles if tables[a]["cls"] == 1]
    assert len(clsX) <= 4 * APC and len(clsY) <= 4 * APC

    def assign(lst, ncores):
        groups = [lst[i::ncores] for i in range(ncores)]
        return [g + [g[-1]] * (APC - len(g)) if g else [dev_angles[0]] * APC
                for g in groups]

    core_angles = assign(clsX, 4) + assign(clsY, 4)

    imgT = np.ascontiguousarray(img.T)
    in_maps, metas = [], []
    for ci in range(NCORES):
        layout = img if ci < 4 else imgT
        im, meta = _core_inputs(layout, core_angles[ci], tables)
        in_maps.append(im)
        metas.append(meta)

    nc = _build_program()
    from concourse import bass_utils
    import os
    trace = bool(int(os.environ.get("CT_TRACE", "0")))
    res = bass_utils.run_bass_kernel_spmd(nc, in_maps,
                                          core_ids=list(range(NCORES)),
                                          trace=trace)
    _PROGRAM_CACHE["exec_time_ns"] = getattr(res, "exec_time_ns", None)
    _PROGRAM_CACHE["last_results"] = res

    proj = np.zeros((NTHETA, Nu), dtype=np.float64)
    done = set()
    for ci in range(NCORES):
        R = res.results[ci]["r_out"]  # [APC, VP, NCHUNK, RW]
        for ai, m in enumerate(metas[ci]):
            a = m["angle"]
            if a in done:
                continue
            done.add(a)
            Mv = m["W"] + 3
            for k in range(NCHUNK):
                base = m["bxi_min"] + m["beta0"][k]
                Rk = R[ai, :, k].astype(np.float64)
                for v in range(VP):
                    n0 = base + v
                    if n0 >= Nu:
                        break
                    hi = min(Mv, Nu - n0)
                    proj[a, n0:n0 + hi] += Rk[v, :hi]

    if rect_angles:
        proj[rect_angles] = _host_project(img, theta[rect_angles])
    return proj.astype(np.float32)



# revision 6
# speedup vs baseline: 1.4059x; 1.4059x over previous
"""CT parallel-beam 2D forward projector on 8 Trainium2 NeuronCores.

v4: W-difference fields + dual-stream gather + v-shifted one-hot binning.

Per view angle the 4 trapezoid tap weights are the difference fields
  W0 = Phi1, W1 = Phi2-Phi1, W2 = Phi3-Phi2, W3 = 1-Phi3
of the cumulative footprint Phi_i = Phic(i-0.5-g), g = f_xi(x)+f_eta(y).
The stacked [512, 4*512] field matrix factors (rank J=64, fp16) per angle;
the device reconstructs pair-interleaved (W0|W1) and (W2|W3) fields with
four 512-col PE matmuls per 128-row chunk.

Device pipeline per (angle, 128-row eta-chunk):
  PE  : psA=(W0|W1), psB=(W2|W3) col-interleaved fields in PSUM fp32
  DVE : pairA words = psA * img2 in ONE fused TT (PSUM fp32 x bf16 -> bf16)
  ACT : psB -> bf16 staging;  DVE: pairB words = staging * img2 (2x mode)
  POOL: ONE indirect_copy moves fp32 words (bf16 field pairs) for BOTH
        pairs through a dual stream: 512 A-slots (first pixel of each
        xi-bin) + 160 B-slots (second pixel of L==2 bins, compacted)
  PE  : 8 binning matmuls, one per (pair-field, A/B region).  The tap
        shift s rides on the PSUM PARTITION axis via v-shifted one-hots
        (lhs = column-sliced views of one [128,102] one-hot buffer), so
        every matmul is a single-piece 512-col (A) or 160-col (B) write.
  ACT : R psum [99, 672] -> SBUF, DMA out
Host: per-(angle,chunk) collapse proj[n] += R[v',m] at n = bxi_min +
beta0 + v' + m (A) / n = ... + l2[j] (B), plus numpy path for the two
degenerate axis-aligned angles.

SPMD: one program for all 8 cores. Cores 0-3 process "class X" angles
(|cos| >= sin) on img; cores 4-7 process "class Y" angles on img.T. All
per-angle variation (SVD factors, one-hots, gather streams) is input data.
"""

import numpy as np

Nx = Ny = 512
Nu = 768
NTHETA = 180
HALF_U = (Nu - 1) / 2.0
NCORES = 8
import os as _os
APC = int(_os.environ.get("CT_APC", "23"))   # angles per core
NCHUNK = 4        # eta chunks of 128
VP = 96           # local v bins per chunk (beta span < 92)
VPX = VP + 3      # v' rows incl. tap shift 0..3
OHW = VPX + 3     # one-hot buffer width (slices [3-f : 3-f+VPX])
AW = 512          # A-region bins (W <= 512 always)
BW = 160          # B-region slots (#L2 <= 150)
PW = AW + BW      # per-pair gather width / R width
MPAD = PW         # gather output words per pair (= 672 = 16*42)
SRCW = 520        # gather source words per pair: fields 0:512, zeros 512:520
ZW = 512          # zero word index
J = int(_os.environ.get("CT_J", "64"))  # SVD rank per angle
B_RECT = 1e-4     # below this min-slope, use the host rect path

_PROGRAM_CACHE = {}


def _np_bf16():
    import ml_dtypes
    return ml_dtypes.bfloat16


# --------------------------------------------------------------------------
# host tables
# --------------------------------------------------------------------------

def _angle_tables(theta_val):
    th = float(theta_val)
    c, s = np.cos(th), np.sin(th)
    ac, asn = abs(c), abs(s)
    A, B = max(ac, asn), min(ac, asn)
    cls = 0 if ac >= asn else 1
    a_xi, a_eta = (c, s) if cls == 0 else (s, c)
    z0 = HALF_U - (ac + asn) / 2 - 255.5 * (c + s)
    grid = np.arange(512)
    pxi = a_xi * grid + z0
    bxi = np.floor(pxi).astype(np.int64)
    fxi = pxi - bxi
    peta = a_eta * grid
    beta = np.floor(peta).astype(np.int64)
    feta = peta - beta
    rect = B <= B_RECT
    return dict(A=A, B=B, cls=cls, rect=rect,
                bxi=bxi, fxi=fxi, beta=beta, feta=feta)


def _gather_tables(T):
    """A-stream first-pixel indices xa and L==2 bin list l2."""
    bxi = T["bxi"]
    bxi_min = int(bxi.min())
    mloc = bxi - bxi_min
    W = int(mloc.max()) + 1
    assert W <= AW
    order = np.argsort(mloc, kind="stable")
    sorted_m = mloc[order]
    first = np.searchsorted(sorted_m, np.arange(W), side="left")
    last = np.searchsorted(sorted_m, np.arange(W), side="right")
    L = last - first
    assert L.min() >= 1 and L.max() <= 2
    xa = order[first]          # first xi of each bin (order is stable)
    l2 = np.where(L == 2)[0]
    assert len(l2) <= BW, len(l2)
    return dict(bxi_min=bxi_min, W=W, xa=xa, l2=l2)


def _wrap_idx(stream):
    """[MPAD] int -> [128, MPAD//16] uint16 wrapped per 16-partition groups."""
    w = stream.reshape(MPAD // 16, 16).T.astype(np.uint16)
    return np.tile(w, (8, 1))


def _phic(t, A, B):
    q = 1.0 / (2 * A * B)
    r = lambda x: np.square(np.maximum(x, 0.0))
    return q * (r(t) - r(t - B) - r(t - A) + r(t - A - B))


def _angle_factors(T):
    """Rank-J factorization of the stacked W-difference fields.

    Returns ByT [J, 512] fp16 and Bx [J, 2048] fp16 with columns permuted
    so device psA cols are (W0|W1) lane-interleaved (xi 0:256 then
    256:512) and psB cols likewise (W2|W3).
    """
    A, B = T["A"], T["B"]
    g = T["feta"][:, None] + T["fxi"][None, :]          # [512, 512]
    P1 = _phic(0.5 - g, A, B)
    P2 = _phic(1.5 - g, A, B)
    P3 = _phic(2.5 - g, A, B)
    M = np.concatenate([P1, P2 - P1, P3 - P2, 1.0 - P3], axis=1)  # [512,2048]
    MMt = M @ M.T
    w, V = np.linalg.eigh(MMt)
    order = np.argsort(w)[::-1][:J]
    w = np.maximum(w[order], 1e-20)
    V = V[:, order]                                     # [512, J]
    s = np.sqrt(np.sqrt(w))                             # sigma^(1/2)
    ByT = (V * s[None, :]).T                            # [J, 512]
    Bx = (V / s[None, :]).T @ M                         # [J, 2048]
    # device col c (global block b = c//512, j = c%512):
    #   xi = (b%2)*256 + j//2, field f = (b//2)*2 + j%2 -> M col f*512+xi
    cg = np.arange(2048)
    b, jj = cg // 512, cg % 512
    perm = ((b // 2) * 2 + jj % 2) * 512 + (b % 2) * 256 + jj // 2
    Bx = Bx[:, perm]
    return ByT.astype(np.float16), Bx.astype(np.float16)


def _core_inputs(img_layout, angle_list, tables):
    """Build the input map for one core. img_layout: [512,512] f32 [eta,xi]."""
    bf16 = _np_bf16()
    A_ = APC
    img = np.ascontiguousarray(img_layout).astype(np.float32)
    imgc = img.reshape(NCHUNK, 128, 512)
    img2 = np.repeat(imgc, 2, axis=2)                   # [4,128,1024]

    ByT_t = np.zeros((A_, J, 512), dtype=np.float16)
    Bx_t = np.zeros((A_, J, 2048), dtype=np.float16)
    oh_t = np.zeros((A_, 128, NCHUNK * OHW), dtype=np.float32)
    idx_t = np.zeros((A_, 128, MPAD // 16), dtype=np.uint16)
    meta = []
    for ai, a in enumerate(angle_list):
        T = tables[a]
        G = _gather_tables(T)
        ByT, Bx = _angle_factors(T)
        ByT_t[ai] = ByT
        Bx_t[ai] = Bx
        beta = T["beta"]
        beta0 = []
        for k in range(NCHUNK):
            sl = slice(k * 128, (k + 1) * 128)
            vloc = beta[sl] - beta[sl].min()
            assert vloc.min() >= 0 and vloc.max() < VP
            oh_t[ai, np.arange(128), k * OHW + 3 + vloc] = 1.0
            beta0.append(int(beta[sl].min()))
        W, xa, l2 = G["W"], G["xa"], G["l2"]
        sA = np.full(AW, ZW, dtype=np.int64)
        sA[:W] = xa[:W]
        sB = np.full(BW, ZW, dtype=np.int64)
        if len(l2):
            sB[:len(l2)] = xa[l2] + 1
        stream = np.concatenate([sA, sB])
        idx_t[ai] = _wrap_idx(stream)
        meta.append(dict(angle=a, bxi_min=G["bxi_min"], W=W, l2=l2,
                         beta0=beta0))
    in_map = {
        "img2_t": img2.astype(bf16),
        "ByT_t": ByT_t,
        "Bx_t": Bx_t,
        "oh_t": oh_t.astype(bf16),
        "idx_t": idx_t,
    }
    return in_map, meta


# --------------------------------------------------------------------------
# the bass program (identical for all cores)
# --------------------------------------------------------------------------

def _build_program():
    if "nc" in _PROGRAM_CACHE:
        return _PROGRAM_CACHE["nc"]

    import concourse.bass as bass
    import concourse.tile as tile
    from concourse import bacc, mybir
    from contextlib import ExitStack

    dt = mybir.dt
    ALU = mybir.AluOpType

    nc = bacc.Bacc("TRN2", target_bir_lowering=False, debug=False,
                   num_devices=NCORES)

    img2_t = nc.dram_tensor("img2_t", [NCHUNK, 128, 1024], dt.bfloat16,
                            kind="ExternalInput").ap()
    ByT_t = nc.dram_tensor("ByT_t", [APC, J, 512], dt.float16,
                           kind="ExternalInput").ap()
    Bx_t = nc.dram_tensor("Bx_t", [APC, J, 2048], dt.float16,
                          kind="ExternalInput").ap()
    oh_t = nc.dram_tensor("oh_t", [APC, 128, NCHUNK * OHW], dt.bfloat16,
                          kind="ExternalInput").ap()
    idx_t = nc.dram_tensor("idx_t", [APC, 128, MPAD // 16], dt.uint16,
                           kind="ExternalInput").ap()
    r_out = nc.dram_tensor("r_out", [APC, NCHUNK, VPX, PW], dt.float32,
                           kind="ExternalOutput").ap()

    with tile.TileContext(nc) as tc, ExitStack() as ctx:
        img_pool = ctx.enter_context(tc.tile_pool(name="img", bufs=1))
        src_pool = ctx.enter_context(tc.tile_pool(name="src", bufs=1))
        stg_pool = ctx.enter_context(tc.tile_pool(name="stg", bufs=1))
        tab_pool = ctx.enter_context(tc.tile_pool(name="tabs", bufs=2))
        g_pool = ctx.enter_context(tc.tile_pool(name="gath", bufs=1))
        o_pool = ctx.enter_context(tc.tile_pool(name="outs", bufs=1))
        psa_pool = ctx.enter_context(tc.tile_pool(name="psumA", bufs=1,
                                                  space="PSUM"))
        psb_pool = ctx.enter_context(tc.tile_pool(name="psumB", bufs=1,
                                                  space="PSUM"))
        psr_pool = ctx.enter_context(tc.tile_pool(name="psumR", bufs=2,
                                                  space="PSUM"))

        img2_ch, srcs = [], []
        for k in range(NCHUNK):
            t = img_pool.tile([128, 1024], dt.bfloat16, tag=f"img2c{k}")
            nc.sync.dma_start(t[:], img2_t[k])
            img2_ch.append(t)
            sr = src_pool.tile([128, 4 * SRCW], dt.bfloat16, tag=f"src{k}")
            nc.vector.memset(sr[:, 1024:1040], 0.0)
            nc.vector.memset(sr[:, 2064:2080], 0.0)
            srcs.append(sr)

        for ai in range(APC):
            idxt = tab_pool.tile([128, MPAD // 16], dt.uint16, tag="idx")
            nc.sync.dma_start(idxt[:], idx_t[ai])
            bx = tab_pool.tile([J, 2048], dt.float16, tag="bx")
            nc.sync.dma_start(bx[:], Bx_t[ai])
            byt = tab_pool.tile([J, 512], dt.float16, tag="byt")
            nc.sync.dma_start(byt[:], ByT_t[ai])
            oht = tab_pool.tile([128, NCHUNK * OHW], dt.bfloat16, tag="oh")
            nc.sync.dma_start(oht[:], oh_t[ai])

            for k in range(NCHUNK):
                byk = byt[:, 128 * k:128 * (k + 1)]
                src = srcs[k]

                psA = psa_pool.tile([128, 1024], dt.float32, tag="psA")
                nc.tensor.matmul(psA[:, 0:512], byk, bx[:, 0:512],
                                 start=True, stop=True)
                nc.tensor.matmul(psA[:, 512:1024], byk, bx[:, 512:1024],
                                 start=True, stop=True)
                psB = psb_pool.tile([128, 1024], dt.float32, tag="psB")
                nc.tensor.matmul(psB[:, 0:512], byk, bx[:, 1024:1536],
                                 start=True, stop=True)
                nc.tensor.matmul(psB[:, 512:1024], byk, bx[:, 1536:2048],
                                 start=True, stop=True)

                # pair A: fused PSUM*img -> bf16 on DVE
                nc.vector.tensor_tensor(src[:, 0:1024], psA[:],
                                        img2_ch[k][:], ALU.mult)
                # pair B: ACT drain to bf16, then DVE 2x mult
                stg = stg_pool.tile([128, 1024], dt.bfloat16, tag=f"stg{k}")
                nc.scalar.copy(stg[:], psB[:])
                nc.vector.tensor_tensor(src[:, 1040:2064], stg[:],
                                        img2_ch[k][:], ALU.mult)

                gtA = g_pool.tile([128, MPAD], dt.float32, tag=f"gtA{k}")
                nc.gpsimd.indirect_copy(
                    gtA[:], src[:, 0:2 * SRCW].bitcast(dt.float32),
                    idxt[:], True)
                gtB = g_pool.tile([128, MPAD], dt.float32, tag=f"gtB{k}")
                nc.gpsimd.indirect_copy(
                    gtB[:], src[:, 2 * SRCW:4 * SRCW].bitcast(dt.float32),
                    idxt[:], True)

                vA = gtA[:].bitcast(dt.bfloat16).rearrange(
                    "p (w l) -> p w l", l=2)
                vB = gtB[:].bitcast(dt.bfloat16).rearrange(
                    "p (w l) -> p w l", l=2)
                lanesA = [vA[:, 0:AW, 0], vA[:, 0:AW, 1],
                          vB[:, 0:AW, 0], vB[:, 0:AW, 1]]
                lanesB = [vA[:, AW:PW, 0], vA[:, AW:PW, 1],
                          vB[:, AW:PW, 0], vB[:, AW:PW, 1]]

                ps = psr_pool.tile([VPX, PW], dt.float32, tag="ps")
                for f in range(4):
                    lhs = oht[:, k * OHW + 3 - f: k * OHW + 3 - f + VPX]
                    nc.tensor.matmul(ps[:, 0:AW], lhs, lanesA[f],
                                     start=(f == 0), stop=(f == 3))
                    nc.tensor.matmul(ps[:, AW:PW], lhs, lanesB[f],
                                     start=(f == 0), stop=(f == 3))

                rout = o_pool.tile([VPX, PW], dt.float32, tag=f"rout{k}")
                nc.scalar.copy(rout[:], ps[:])
                nc.sync.dma_start(r_out[ai, k], rout[:])

    nc.compile()
    _PROGRAM_CACHE["nc"] = nc
    return nc


# --------------------------------------------------------------------------
# host-side rect path (degenerate angles) — numpy port of the reference
# --------------------------------------------------------------------------

def _host_project(img, theta_vals):
    y = (np.arange(Ny) - (Ny - 1) / 2.0)
    x = (np.arange(Nx) - (Nx - 1) / 2.0)
    y2d, x2d = np.meshgrid(y, x, indexing="ij")
    img_v = img.reshape(-1).astype(np.float64)
    out = np.zeros((len(theta_vals), Nu), dtype=np.float64)
    K = 4
    for t, th in enumerate(theta_vals):
        th = float(th)
        cos_t, sin_t = np.cos(th), np.sin(th)
        ac, asn = abs(cos_t), abs(sin_t)
        h = min(1.0 / ac if ac > 0 else np.inf, 1.0 / asn if asn > 0 else np.inf)
        b1 = abs(asn - ac)
        b2 = abs(asn + ac)
        u0 = x2d * cos_t + y2d * sin_t
        u1 = u0 - b2 / 2
        u2 = u0 - b1 / 2
        u3 = u0 + b1 / 2
        u4 = u0 + b2 / 2
        base = np.floor(u1 + HALF_U).astype(np.int64)
        den12 = (u2 - u1) + (u1 == u2)
        den34 = (u4 - u3) + (u3 == u4)
        acc = np.zeros(Nu + 8, dtype=np.float64)
        for k in range(K):
            idx = base + k
            u = idx - HALF_U
            lo, hi = u - 0.5, u + 0.5
            uA = np.maximum(u1, lo); uB = np.minimum(u2, hi)
            w = (uB > uA) * (h / (2.0 * den12)) * ((uB - u1) ** 2 - (uA - u1) ** 2)
            uA = np.maximum(u2, lo); uB = np.minimum(u3, hi)
            w = w + (uB > uA) * h * (uB - uA)
            uA = np.maximum(u3, lo); uB = np.minimum(u4, hi)
            w = w + (uB > uA) * (h / (2.0 * den34)) * ((uA - u4) ** 2 - (uB - u4) ** 2)
            np.add.at(acc, np.clip(idx.reshape(-1), 0, Nu - 1),
                      img_v * w.reshape(-1))
        out[t] = acc[:Nu]
    return out.astype(np.float32)


# --------------------------------------------------------------------------
# main entry
# --------------------------------------------------------------------------

def kernel(img, theta):
    img = np.asarray(img, dtype=np.float32)
    theta = np.asarray(theta, dtype=np.float32)
    assert img.shape == (Ny, Nx) and theta.shape == (NTHETA,)

    tables = {a: _angle_tables(theta[a]) for a in range(NTHETA)}
    rect_angles = [a for a in range(NTHETA) if tables[a]["rect"]]
    dev_angles = [a for a in range(NTHETA) if not tables[a]["rect"]]
    clsX = [a for a in dev_angles if tables[a]["cls"] == 0]
    clsY = [a for a in dev_angles if tables[a]["cls"] == 1]
    assert len(clsX) <= 4 * APC and len(clsY) <= 4 * APC

    def assign(lst, ncores):
        groups = [lst[i::ncores] for i in range(ncores)]
        return [g + [g[-1]] * (APC - len(g)) if g else [dev_angles[0]] * APC
                for g in groups]

    core_angles = assign(clsX, 4) + assign(clsY, 4)

    imgT = np.ascontiguousarray(img.T)
    in_maps, metas = [], []
    for ci in range(NCORES):
        layout = img if ci < 4 else imgT
        im, meta = _core_inputs(layout, core_angles[ci], tables)
        in_maps.append(im)
        metas.append(meta)

    nc = _build_program()
    from concourse import bass_utils
    import os
    trace = bool(int(os.environ.get("CT_TRACE", "0")))
    res = bass_utils.run_bass_kernel_spmd(nc, in_maps,
                                          core_ids=list(range(NCORES)),
                                          trace=trace)
    _PROGRAM_CACHE["exec_time_ns"] = getattr(res, "exec_time_ns", None)
    _PROGRAM_CACHE["last_results"] = res

    OFF = 128
    acc = np.zeros(OFF + Nu + OFF + AW, dtype=np.float64)
    proj = np.zeros((NTHETA, Nu), dtype=np.float64)
    vrows = np.arange(VPX)
    done = set()
    for ci in range(NCORES):
        R = res.results[ci]["r_out"]  # [APC, NCHUNK, VPX, PW]
        for ai, m in enumerate(metas[ci]):
            a = m["angle"]
            if a in done:
                continue
            done.add(a)
            acc[:] = 0.0
            l2 = m["l2"]
            nB = len(l2)
            for k in range(NCHUNK):
                n0 = OFF + m["bxi_min"] + m["beta0"][k]
                Rk = R[ai, k].astype(np.float64)
                for vq in range(VPX):
                    acc[n0 + vq: n0 + vq + AW] += Rk[vq, :AW]
                if nB:
                    np.add.at(acc, (n0 + vrows)[:, None] + l2[None, :],
                              Rk[:, AW:AW + nB])
            proj[a] = acc[OFF:OFF + Nu]

    if rect_angles:
        proj[rect_angles] = _host_project(img, theta[rect_angles])
    return proj.astype(np.float32)
